# Initial kernel scaffold
#
"""Distributed GATv2 message-passing kernel for 8 Trainium2 NeuronCores.

Sharding: nodes (and their incoming edges) are partitioned across the 8
cores by dst-node chunk of 1024; GATv2 weights are replicated; node
features are exchanged once per layer with a bf16 AllGather.

Per core / per layer on device:
  - xr = x_own @ Wr + br           (node-space matmul, lhsT = x^T)
  - rel_proj = rel_emb @ We + bl   (64 relations only)
  - per-edge src features gathered (transposed) with dma_gather, then
    m = X_src@Wl + onehotR@rel_proj + onehotD@xr accumulated in PSUM
  - logits = 0.2*att.m + 0.8*att.relu(m); the linear att.m term is
    computed exactly in node space via host-folded v=W.att weights
  - segment softmax + weighted aggregation via one-hot matmuls (the
    edge->node-tile assignment matrices are shipped as static data)
  - head mean + gelu + residual; bf16 chunk AllGather for next layer
Final: gate/fuse with x_text, Wp projection, layernorm, gelu.

Edges are sorted by dst and packed into 128-wide tiles grouped by
128-node dst tile; tiles-per-group is padded to a global constant so one
SPMD program serves all 8 cores (per-core differences live in the data:
gather indices and one-hot matrices).
"""
import sys
sys.path.insert(0, "/opt/trn_rl_repo")

import numpy as np
import ml_dtypes

import concourse.bass as bass
import concourse.bacc as bacc
import concourse.mybir as mybir
import concourse.tile as tile
from concourse import library_config
from concourse.bass_utils import run_bass_kernel_spmd

AF = mybir.ActivationFunctionType
OP = mybir.AluOpType
dt = mybir.dt
AX = mybir.AxisListType

N, D, E, L, H, R = 8192, 768, 16384, 3, 4, 64
NC = 8            # cores
CHN = N // NC     # 1024 nodes per core
NT = CHN // 128   # 8 node tiles per core
HD = H * D        # 3072
KT = D // 128     # 6 contraction tiles
NCH = 8           # column chunks of 384 (2 per head)
CW = HD // NCH    # 384
EPS_LN = 1e-5
DEN_EPS = 1e-6

bf16 = ml_dtypes.bfloat16


# ---------------------------------------------------------------- host prep
def _preprocess(x_text, rel_emb, Wl, bl, Wr, br, We, att, bout,
                Wg, bg, Wp, bp, gamma, beta, edge_index, edge_attr):
    src_all = np.asarray(edge_index[0], np.int64)
    dst_all = np.asarray(edge_index[1], np.int64)
    rel_all = np.asarray(edge_attr, np.int64)

    per_core = []
    max_tiles = 1
    for c in range(NC):
        sel = np.nonzero((dst_all >= c * CHN) & (dst_all < (c + 1) * CHN))[0]
        order = np.argsort(dst_all[sel], kind="stable")
        sel = sel[order]
        dloc = dst_all[sel] - c * CHN
        groups = []
        for g in range(NT):
            gsel = sel[(dloc >= g * 128) & (dloc < (g + 1) * 128)]
            groups.append(gsel)
            max_tiles = max(max_tiles, (len(gsel) + 127) // 128)
        per_core.append(groups)
    TG = max_tiles               # tiles per node-tile group (uniform)
    ET = NT * TG                 # edge tiles per core
    EP = ET * 128                # padded edges per core

    in_maps = []
    w_shared = None
    for c in range(NC):
        src_idx = np.zeros(EP, np.int16)
        oneD = np.zeros((128, ET, 128), bf16)   # [dst_local, tile, e]
        oneA = np.zeros((128, ET, 128), bf16)   # [e, tile, dst_local]
        oneR = np.zeros((R, ET, 128), bf16)     # [rel, tile, e]
        for g in range(NT):
            ge = per_core[c][g]
            for i, eidx in enumerate(ge):
                t = g * TG + i // 128
                e = i % 128
                nl = int(dst_all[eidx]) - c * CHN - g * 128
                src_idx[t * 128 + e] = int(src_all[eidx])
                oneD[nl, t, e] = 1
                oneA[e, t, nl] = 1
                oneR[int(rel_all[eidx]), t, e] = 1
        idx_w = np.tile(src_idx.reshape(EP // 16, 16).T, (8, 1)).copy()

        if w_shared is None:
            # weight tensors, cast/reshaped only (shared by all cores)
            def ktile(w):  # [768, X] -> [128, 6, X]
                return np.ascontiguousarray(
                    w.reshape(KT, 128, -1).transpose(1, 0, 2)).astype(bf16)
            wl_t = np.stack([ktile(np.asarray(Wl[l])) for l in range(L)])
            wr_t = np.stack([ktile(np.asarray(Wr[l])) for l in range(L)])
            we_t = np.stack([ktile(np.asarray(We[l])) for l in range(L)])
            relT = ktile(np.asarray(rel_emb).T)          # [128, 6, 64]
            wg_t = np.ascontiguousarray(
                np.asarray(Wg).reshape(12, 128, 1).transpose(1, 0, 2)).astype(bf16)
            wp_t = ktile(np.asarray(Wp))                 # [128, 6, 768]
            attm = np.asarray(att).reshape(L, HD)        # [L, 3072]
            att_rep = np.broadcast_to(attm[:, None, :], (L, 128, HD)).astype(bf16)
            # v = per-head fold of att into Wl / Wr (weight algebra only)
            # vl[l][d, h] = sum_j Wl[l][d, h*D+j] * att[l,h,j]
            vl = np.stack([
                np.einsum("dhj,hj->dh", np.asarray(Wl[l]).reshape(D, H, D),
                          np.asarray(att[l])) for l in range(L)])
            vr = np.stack([
                np.einsum("dhj,hj->dh", np.asarray(Wr[l]).reshape(D, H, D),
                          np.asarray(att[l])) for l in range(L)])
            vl_t = np.stack([np.ascontiguousarray(
                v.reshape(KT, 128, H).transpose(1, 0, 2)).astype(bf16) for v in vl])
            vr_t = np.stack([np.ascontiguousarray(
                v.reshape(KT, 128, H).transpose(1, 0, 2)).astype(bf16) for v in vr])
            # att.br constant per head (br is [3072])
            cbr = np.einsum("lhj,lhj->lh", np.asarray(att),
                            np.asarray(br).reshape(L, H, D))
            cbr_rep = np.broadcast_to(
                cbr[:, None, :], (L, 128, H)).astype(np.float32)
            bl_rep = np.broadcast_to(
                np.asarray(bl)[:, None, :], (L, 128, HD)).astype(np.float32)
            br_rep = np.broadcast_to(
                np.asarray(br)[:, None, :], (L, 128, HD)).astype(bf16)
            bout_rep = np.broadcast_to(
                np.asarray(bout)[:, None, :], (L, 128, D)).astype(np.float32)
            bg_rep = np.full((128, 1), float(np.asarray(bg)[0]), np.float32)
            bp_rep = np.broadcast_to(
                np.asarray(bp)[None, :], (128, D)).astype(np.float32)
            gamma_rep = np.broadcast_to(
                np.asarray(gamma)[None, :], (128, D)).astype(np.float32)
            beta_rep = np.broadcast_to(
                np.asarray(beta)[None, :], (128, D)).astype(np.float32)
            x_full_bf = np.asarray(x_text, np.float32).astype(bf16)
            w_shared = dict(
                wl=np.ascontiguousarray(wl_t), wr=np.ascontiguousarray(wr_t),
                we=np.ascontiguousarray(we_t), relT=np.ascontiguousarray(relT),
                wg=np.ascontiguousarray(wg_t), wp=np.ascontiguousarray(wp_t),
                att_rep=np.ascontiguousarray(att_rep),
                vl=vl_t, vr=vr_t, cbr=np.ascontiguousarray(cbr_rep),
                bl_rep=np.ascontiguousarray(bl_rep),
                br_rep=np.ascontiguousarray(br_rep),
                bout_rep=np.ascontiguousarray(bout_rep),
                bg_rep=bg_rep, bp_rep=np.ascontiguousarray(bp_rep),
                gamma_rep=np.ascontiguousarray(gamma_rep),
                beta_rep=np.ascontiguousarray(beta_rep),
                x_full=np.ascontiguousarray(x_full_bf),
            )
        x_own = np.ascontiguousarray(np.asarray(
            x_text[c * CHN:(c + 1) * CHN], np.float32))
        m = dict(w_shared)
        m.update(x_own=x_own, x_own_bf=x_own.astype(bf16),
                 src_idx=idx_w, oneD=oneD, oneA=oneA, oneR=oneR)
        in_maps.append(m)
    return in_maps, TG


# ---------------------------------------------------------------- device
def build_program(TG, repeat=1, abl=frozenset()):
    ET = NT * TG
    EP = ET * 128
    nc = bacc.Bacc("TRN2", target_bir_lowering=False, debug=False,
                   num_devices=NC)

    def inp(name, shape, dtype):
        return nc.dram_tensor(name, list(shape), dtype, kind="ExternalInput")

    x_full = inp("x_full", [N, D], dt.bfloat16)
    x_own = inp("x_own", [CHN, D], dt.float32)
    x_own_bf = inp("x_own_bf", [CHN, D], dt.bfloat16)
    wl_d = inp("wl", [L, 128, KT, HD], dt.bfloat16)
    wr_d = inp("wr", [L, 128, KT, HD], dt.bfloat16)
    we_d = inp("we", [L, 128, KT, HD], dt.bfloat16)
    relT_d = inp("relT", [128, KT, R], dt.bfloat16)
    wg_d = inp("wg", [128, 12, 1], dt.bfloat16)
    wp_d = inp("wp", [128, KT, D], dt.bfloat16)
    att_d = inp("att_rep", [L, 128, HD], dt.bfloat16)
    vl_d = inp("vl", [L, 128, KT, H], dt.bfloat16)
    vr_d = inp("vr", [L, 128, KT, H], dt.bfloat16)
    cbr_d = inp("cbr", [L, 128, H], dt.float32)
    blr_d = inp("bl_rep", [L, 128, HD], dt.bfloat16)
    brr_d = inp("br_rep", [L, 128, HD], dt.bfloat16)
    bor_d = inp("bout_rep", [L, 128, D], dt.float32)
    bgr_d = inp("bg_rep", [128, 1], dt.float32)
    bpr_d = inp("bp_rep", [128, D], dt.float32)
    gmr_d = inp("gamma_rep", [128, D], dt.float32)
    btr_d = inp("beta_rep", [128, D], dt.float32)
    idx_d = inp("src_idx", [128, EP // 16], dt.int16)
    oneD_d = inp("oneD", [128, ET, 128], dt.bfloat16)
    oneA_d = inp("oneA", [128, ET, 128], dt.bfloat16)
    oneR_d = inp("oneR", [R, ET, 128], dt.bfloat16)

    out_d = nc.dram_tensor("out", [CHN, D], dt.float32, kind="ExternalOutput")

    xg_chunk = nc.dram_tensor("xg_chunk", [CHN, D], dt.bfloat16)
    xc_dram = nc.dram_tensor("xc_dram", [CHN, D], dt.float32)
    xg_full = nc.dram_tensor("xg_full", [N, D], dt.bfloat16,
                             addr_space="Shared")
    xf_dram = nc.dram_tensor("xf_dram", [CHN, D], dt.bfloat16)

    x_own_t = x_own.rearrange("(t p) d -> p t d", p=128)
    xc_dram_t = xc_dram.rearrange("(t p) d -> p t d", p=128)

    with tile.TileContext(nc) as tc:
        nc.gpsimd.load_library(library_config.attnmlp)
        with tc.tile_pool(name="persist", bufs=1) as pp, \
             tc.tile_pool(name="scr", bufs=2) as sp, \
             tc.tile_pool(name="pm", bufs=3, space="PSUM") as pm, \
             tc.tile_pool(name="pt", bufs=2, space="PSUM") as pt, \
             tc.tile_pool(name="pagg", bufs=2, space="PSUM") as pagg:

            idxs = pp.tile([128, EP // 16], dt.int16)
            nc.sync.dma_start(out=idxs[:], in_=idx_d[:])

            for _rep in range(repeat):
                with tc.tile_pool(name="work", bufs=1) as wk:
                  for l in range(L):
                    XF = x_full if l == 0 else xg_full
                    XCSRC = x_own_bf if l == 0 else xg_chunk
                    XMST = x_own_t if l == 0 else xc_dram_t

                    wl_s = wk.tile([128, KT, HD], dt.bfloat16, tag="wl_s")
                    nc.sync.dma_start(out=wl_s[:], in_=wl_d[l])
                    wr_s = wk.tile([128, KT, HD], dt.bfloat16, tag="wr_s")
                    nc.sync.dma_start(out=wr_s[:], in_=wr_d[l])
                    attr = wk.tile([128, HD], dt.bfloat16, tag="attr")
                    nc.sync.dma_start(out=attr[:], in_=att_d[l])
                    brr = wk.tile([128, HD], dt.bfloat16, tag="brr")
                    nc.sync.dma_start(out=brr[:], in_=brr_d[l])
                    blr = wk.tile([128, HD], dt.bfloat16, tag="blr")
                    nc.sync.dma_start(out=blr[:], in_=blr_d[l])
                    bor = wk.tile([128, D], dt.float32, tag="bor")
                    nc.sync.dma_start(out=bor[:], in_=bor_d[l])
                    vl_s = wk.tile([128, KT, H], dt.bfloat16, tag="vl_s")
                    nc.sync.dma_start(out=vl_s[:], in_=vl_d[l])
                    vr_s = wk.tile([128, KT, H], dt.bfloat16, tag="vr_s")
                    nc.sync.dma_start(out=vr_s[:], in_=vr_d[l])
                    cbr_s = wk.tile([128, H], dt.float32, tag="cbr_s")
                    nc.sync.dma_start(out=cbr_s[:], in_=cbr_d[l])

                    # blmean = 0.25 * sum_h bl[h]
                    blm = wk.tile([128, D], dt.float32, tag="blm")
                    nc.vector.tensor_tensor(out=blm[:], in0=blr[:, 0:D],
                                            in1=blr[:, D:2 * D], op=OP.add)
                    nc.vector.tensor_tensor(out=blm[:], in0=blm[:],
                                            in1=blr[:, 2 * D:3 * D], op=OP.add)
                    nc.vector.tensor_tensor(out=blm[:], in0=blm[:],
                                            in1=blr[:, 3 * D:4 * D], op=OP.add)
                    nc.vector.tensor_scalar(out=blm[:], in0=blm[:], scalar1=0.25,
                                            scalar2=None, op0=OP.mult)

                    # rel_proj' = rel_emb @ We + bl  -> [64, 3072] bf16
                    relT_s = wk.tile([128, KT, R], dt.bfloat16, tag="relT_s")
                    nc.sync.dma_start(out=relT_s[:], in_=relT_d[:])
                    relp = wk.tile([R, HD], dt.bfloat16, tag="relp")
                    for ch in range(NCH):
                        sl = slice(ch * CW, (ch + 1) * CW)
                        wech = wk.tile([128, KT, CW], dt.bfloat16, tag="wech")
                        nc.sync.dma_start(out=wech[:], in_=we_d[l][:, :, sl])
                        ps = pm.tile([128, CW], dt.float32, tag="pm")
                        for k in range(KT):
                            nc.tensor.matmul(ps[:R, :], relT_s[:, k, :],
                                             wech[:, k, :],
                                             start=(k == 0), stop=(k == KT - 1))
                        nc.vector.tensor_tensor(out=relp[:, sl], in0=ps[:R, :],
                                                in1=blr[:R, sl], op=OP.add)
                    # a_e[r, h] = att_h . relp'[r, h-block]
                    ae = wk.tile([R, H], dt.float32, tag="ae")
                    ae_bf = wk.tile([R, H], dt.bfloat16, tag="ae_bf")
                    for h in range(H):
                        hb = slice(h * D, (h + 1) * D)
                        scr_ae = sp.tile([R, D], dt.bfloat16, tag="scr_ae")
                        nc.vector.tensor_tensor(out=scr_ae[:], in0=relp[:, hb],
                                                in1=attr[:R, hb], op=OP.mult)
                        nc.vector.tensor_reduce(out=ae[:, h:h + 1], in_=scr_ae[:],
                                                axis=AX.X, op=OP.add)
                    nc.vector.tensor_copy(ae_bf[:], ae[:])

                    for g in range(NT):
                        gs = slice(g * 128, (g + 1) * 128)
                        # x^T for this node tile
                        xT = wk.tile([128, KT, 128], dt.bfloat16, tag="xT",
                                     bufs=2)
                        for k in range(KT):
                            nc.sync.dma_start(
                                out=xT[:, k, :],
                                in_=XCSRC[gs, k * 128:(k + 1) * 128],
                                transpose=True)
                        # xr_g = x_g @ Wr + br ; a_r_g = x_g @ vr + cbr
                        xr = wk.tile([128, HD], dt.bfloat16, tag="xr", bufs=2)
                        if "ph1" in abl:
                            nc.vector.memset(xr[:], 0.0)
                        for ch in (range(NCH) if "ph1" not in abl else []):
                            sl = slice(ch * CW, (ch + 1) * CW)
                            ps = pm.tile([128, CW], dt.float32, tag="pm")
                            for k in range(KT):
                                nc.tensor.matmul(ps[:], xT[:, k, :],
                                                 wr_s[:, k, sl],
                                                 start=(k == 0), stop=(k == KT - 1))
                            nc.vector.tensor_tensor(out=xr[:, sl], in0=ps[:],
                                                    in1=brr[:, sl], op=OP.add)
                        arp = pt.tile([128, H], dt.float32, tag="pt")
                        for k in range(KT):
                            nc.tensor.matmul(arp[:], xT[:, k, :], vr_s[:, k, :],
                                             start=(k == 0), stop=(k == KT - 1))
                        art = sp.tile([128, H], dt.float32, tag="art")
                        nc.vector.tensor_tensor(out=art[:], in0=arp[:],
                                                in1=cbr_s[:], op=OP.add)
                        ar_bf = wk.tile([128, H], dt.bfloat16, tag="ar_bf",
                                        bufs=2)
                        nc.vector.tensor_copy(ar_bf[:], art[:])

                        # per-group static one-hots
                        ts0 = g * TG
                        oneDg = wk.tile([128, TG, 128], dt.bfloat16, tag="oneDg",
                                        bufs=2)
                        nc.sync.dma_start(out=oneDg[:],
                                          in_=oneD_d[:, ts0:ts0 + TG, :])
                        oneAg = wk.tile([128, TG, 128], dt.bfloat16, tag="oneAg",
                                        bufs=2)
                        nc.sync.dma_start(out=oneAg[:],
                                          in_=oneA_d[:, ts0:ts0 + TG, :])
                        oneRg = wk.tile([R, TG, 128], dt.bfloat16, tag="oneRg",
                                        bufs=2)
                        nc.sync.dma_start(out=oneRg[:],
                                          in_=oneR_d[:, ts0:ts0 + TG, :])

                        e0 = g * TG * 128
                        xsT = wk.tile([128, KT, TG * 128], dt.bfloat16,
                                      tag="xsT", bufs=2)
                        if "gather" not in abl:
                            nc.gpsimd.dma_gather(
                                xsT[:], XF[:],
                                idxs[:, e0 // 16:(e0 + TG * 128) // 16],
                                num_idxs=TG * 128, num_idxs_reg=TG * 128,
                                elem_size=D, transpose=True)
                        else:
                            nc.sync.dma_start(
                                out=xsT.rearrange("p a b -> p (a b)"),
                                in_=XF[0:TG * 128, :].rearrange(
                                    "(p a) d -> p (a d)", p=128))
                        xl_sb = wk.tile([128, TG, HD], dt.bfloat16, tag="xl_sb")
                        ex_sb = wk.tile([128, TG, H], dt.bfloat16, tag="ex_sb")
                        exf = wk.tile([128, TG, H], dt.float32, tag="exf")
                        for kt in range(TG):
                            es = slice(kt * 128, (kt + 1) * 128)
                            # linear logit part: x_src@vl + a_r[dst] + a_e[rel]
                            aps = pt.tile([128, H], dt.float32, tag="pt")
                            for k in range(KT):
                                nc.tensor.matmul(aps[:], xsT[:, k, es],
                                                 vl_s[:, k, :],
                                                 start=(k == 0), stop=False)
                            nc.tensor.matmul(aps[:], oneDg[:, kt, :],
                                             ar_bf[:], start=False, stop=False)
                            nc.tensor.matmul(aps[:], oneRg[:, kt, :], ae_bf[:],
                                             start=False, stop=True)
                            racc = sp.tile([128, 2, H], dt.float32, tag="racc")
                            nc.vector.memset(racc[:], 0.0)
                            for ch in (range(NCH) if "medge" not in abl else []):
                                h, j = ch // 2, ch % 2
                                sl = slice(ch * CW, (ch + 1) * CW)
                                mp = pm.tile([128, CW], dt.float32, tag="pm")
                                for k in range(KT):
                                    nc.tensor.matmul(mp[:], xsT[:, k, es],
                                                     wl_s[:, k, sl],
                                                     start=(k == 0),
                                                     stop=(k == KT - 1))
                                nc.vector.tensor_copy(xl_sb[:, kt, sl], mp[:])
                                nc.tensor.matmul(mp[:], oneRg[:, kt, :],
                                                 relp[:, sl], start=False,
                                                 stop=False)
                                nc.tensor.matmul(mp[:], oneDg[:, kt, :],
                                                 xr[:, sl], start=False,
                                                 stop=True)
                                rt = sp.tile([128, CW], dt.bfloat16, tag="rt")
                                nc.scalar.activation(rt[:], mp[:], AF.Relu)
                                sc = sp.tile([128, CW], dt.bfloat16, tag="sc")
                                nc.vector.tensor_tensor(out=sc[:], in0=rt[:],
                                                        in1=attr[:, sl],
                                                        op=OP.mult)
                                nc.vector.tensor_reduce(
                                    out=racc[:, j, h:h + 1], in_=sc[:],
                                    axis=AX.X, op=OP.add)
                            if "medge" in abl:
                                nc.vector.memset(
                                    xl_sb.rearrange("p a b -> p (a b)"), 0.0)
                            # logits = 0.2*aps + 0.8*(racc_j0 + racc_j1)
                            lg = sp.tile([128, H], dt.float32, tag="lg")
                            nc.vector.tensor_tensor(
                                out=lg[:], in0=racc[:, 0, :], in1=racc[:, 1, :],
                                op=OP.add)
                            nc.vector.tensor_scalar(out=lg[:], in0=lg[:],
                                                    scalar1=0.8, scalar2=None,
                                                    op0=OP.mult)
                            lg2 = sp.tile([128, H], dt.float32, tag="lg2")
                            nc.vector.tensor_scalar(out=lg2[:], in0=aps[:],
                                                    scalar1=0.2, scalar2=None,
                                                    op0=OP.mult)
                            nc.vector.tensor_tensor(out=lg[:], in0=lg[:],
                                                    in1=lg2[:], op=OP.add)
                            nc.scalar.activation(exf[:, kt, :], lg[:], AF.Exp)
                            nc.vector.tensor_copy(ex_sb[:, kt, :], exf[:, kt, :])

                        # segment softmax pieces
                        dn = pt.tile([128, H], dt.float32, tag="pt")
                        for kt in range(TG):
                            nc.tensor.matmul(dn[:], oneAg[:, kt, :],
                                             ex_sb[:, kt, :],
                                             start=(kt == 0), stop=(kt == TG - 1))
                        dn4 = sp.tile([128, H], dt.float32, tag="dn4")
                        nc.vector.tensor_scalar(out=dn4[:], in0=dn[:],
                                                scalar1=4.0, scalar2=DEN_EPS,
                                                op0=OP.mult, op1=OP.add)
                        has = sp.tile([128, 1], dt.float32, tag="has")
                        nc.vector.tensor_scalar(out=has[:], in0=dn4[:, 0:1],
                                                scalar1=2.0 * DEN_EPS,
                                                scalar2=None, op0=OP.is_gt)
                        rden = sp.tile([128, H], dt.float32, tag="rden")
                        nc.vector.reciprocal(rden[:], dn4[:])
                        rden_bf = sp.tile([128, H], dt.bfloat16, tag="rden_bf")
                        nc.vector.tensor_copy(rden_bf[:], rden[:])

                        A_sb = wk.tile([128, TG, H, 128], dt.bfloat16, tag="A_sb")
                        if "agg" in abl:
                            nc.vector.memset(
                                A_sb.rearrange("p a b c -> p (a b c)"), 0.0)
                        for kt in (range(TG) if "agg" not in abl else []):
                            re = pt.tile([128, H], dt.float32, tag="pt")
                            nc.tensor.matmul(re[:], oneDg[:, kt, :], rden_bf[:],
                                             start=True, stop=True)
                            re_f = sp.tile([128, H], dt.float32, tag="re_f")
                            nc.vector.tensor_copy(re_f[:], re[:])
                            for h in range(H):
                                nc.vector.tensor_scalar(
                                    out=A_sb[:, kt, h, :], in0=oneAg[:, kt, :],
                                    scalar1=exf[:, kt, h:h + 1],
                                    scalar2=re_f[:, h:h + 1],
                                    op0=OP.mult, op1=OP.mult)

                        # aggregate + head mean (0.25 folded via 4*den)
                        xcg = sp.tile([128, D], dt.float32, tag="escr2", bufs=2)
                        nc.sync.dma_start(out=xcg[:], in_=XMST[:, g, :])
                        for j in range(2):
                            jsl = slice(j * CW, (j + 1) * CW)
                            ag = pagg.tile([128, CW], dt.float32, tag="pagg")
                            if "aggmm" in abl:
                                nc.vector.memset(ag[:], 0.0)
                            first = True
                            for kt in (range(TG) if "aggmm" not in abl else []):
                                for h in range(H):
                                    nc.tensor.matmul(
                                        ag[:], A_sb[:, kt, h, :],
                                        xl_sb[:, kt, h * D + j * CW:
                                              h * D + (j + 1) * CW],
                                        start=first,
                                        stop=(kt == TG - 1 and h == H - 1))
                                    first = False
                            u = sp.tile([128, CW], dt.float32, tag="escr", bufs=5)
                            nc.vector.tensor_tensor(out=u[:], in0=ag[:],
                                                    in1=bor[:, jsl], op=OP.add)
                            bh = sp.tile([128, CW], dt.float32, tag="escr", bufs=5)
                            nc.vector.tensor_scalar(out=bh[:], in0=blm[:, jsl],
                                                    scalar1=has[:],
                                                    scalar2=None, op0=OP.mult)
                            nc.vector.tensor_tensor(out=u[:], in0=u[:],
                                                    in1=bh[:], op=OP.add)
                            gl = sp.tile([128, CW], dt.float32, tag="escr", bufs=5)
                            nc.scalar.activation(gl[:], u[:], AF.Gelu)
                            xn = sp.tile([128, CW], dt.float32, tag="escr", bufs=5)
                            nc.vector.tensor_tensor(out=xn[:], in0=gl[:],
                                                    in1=xcg[:, jsl], op=OP.add)
                            nc.sync.dma_start(out=xc_dram_t[:, g, jsl], in_=xn[:])
                            xnb = sp.tile([128, CW], dt.bfloat16, tag="xnb")
                            nc.vector.tensor_copy(xnb[:], xn[:])
                            nc.sync.dma_start(
                                out=xg_chunk[gs, jsl], in_=xnb[:])

                    if l < L - 1 and "ag" not in abl:
                        nc.gpsimd.collective_compute(
                            "AllGather", OP.bypass,
                            ins=[xg_chunk[:]], outs=[xg_full[:]],
                            replica_groups=[list(range(NC))])

                # ------------- final: gate, fuse, project, layernorm, gelu
                with tc.tile_pool(name="fin", bufs=1) as fp:
                    xgT = fp.tile([128, KT, CHN], dt.bfloat16, tag="xgT")
                    for k in range(KT):
                        nc.sync.dma_start(out=xgT[:, k, :],
                                          in_=xg_chunk[:, k * 128:(k + 1) * 128],
                                          transpose=True)
                    xtT = fp.tile([128, KT, CHN], dt.bfloat16, tag="xtT")
                    for k in range(KT):
                        nc.sync.dma_start(out=xtT[:, k, :],
                                          in_=x_own_bf[:, k * 128:(k + 1) * 128],
                                          transpose=True)
                    wg_s = fp.tile([128, 12, 1], dt.bfloat16, tag="wg_s")
                    nc.sync.dma_start(out=wg_s[:], in_=wg_d[:])
                    bgr = fp.tile([128, 1], dt.float32, tag="bgr")
                    nc.sync.dma_start(out=bgr[:], in_=bgr_d[:])

                    for nt in range(NT):
                        ns = slice(nt * 128, (nt + 1) * 128)
                        pg = pt.tile([128, 1], dt.float32, tag="pt")
                        for k in range(KT):
                            nc.tensor.matmul(pg[:], xtT[:, k, ns], wg_s[:, k, :],
                                             start=(k == 0), stop=False)
                        for k in range(KT):
                            nc.tensor.matmul(pg[:], xgT[:, k, ns],
                                             wg_s[:, KT + k, :],
                                             start=False, stop=(k == KT - 1))
                        alph = fp.tile([128, 1], dt.float32, tag="alph", bufs=2)
                        nc.scalar.activation(alph[:], pg[:], AF.Sigmoid,
                                             bias=bgr[:])
                        xct = fp.tile([128, D], dt.float32, tag="xct", bufs=2)
                        nc.sync.dma_start(out=xct[:], in_=xc_dram_t[:, nt, :])
                        xtt = fp.tile([128, D], dt.float32, tag="xtt", bufs=2)
                        nc.sync.dma_start(out=xtt[:], in_=x_own_t[:, nt, :])
                        dif = fp.tile([128, D], dt.float32, tag="dif", bufs=2)
                        nc.vector.tensor_tensor(out=dif[:], in0=xct[:],
                                                in1=xtt[:], op=OP.subtract)
                        nc.vector.tensor_scalar(out=dif[:], in0=dif[:],
                                                scalar1=alph[:], scalar2=None,
                                                op0=OP.mult)
                        nc.vector.tensor_tensor(out=dif[:], in0=dif[:],
                                                in1=xtt[:], op=OP.add)
                        dif_bf = fp.tile([128, D], dt.bfloat16, tag="dif_bf", bufs=2)
                        nc.vector.tensor_copy(dif_bf[:], dif[:])
                        nc.sync.dma_start(out=xf_dram[ns, :], in_=dif_bf[:])

                    xfT = fp.tile([128, KT, CHN], dt.bfloat16, tag="xgT")
                    for k in range(KT):
                        nc.sync.dma_start(out=xfT[:, k, :],
                                          in_=xf_dram[:, k * 128:(k + 1) * 128],
                                          transpose=True)
                    wp_s = fp.tile([128, KT, D], dt.bfloat16, tag="wp_s")
                    nc.sync.dma_start(out=wp_s[:], in_=wp_d[:])
                    bpr = fp.tile([128, D], dt.float32, tag="bpr")
                    nc.sync.dma_start(out=bpr[:], in_=bpr_d[:])
                    gmr = fp.tile([128, D], dt.float32, tag="gmr")
                    nc.sync.dma_start(out=gmr[:], in_=gmr_d[:])
                    btr = fp.tile([128, D], dt.float32, tag="btr")
                    nc.sync.dma_start(out=btr[:], in_=btr_d[:])

                    for nt in range(NT):
                        ns = slice(nt * 128, (nt + 1) * 128)
                        y = fp.tile([128, D], dt.float32, tag="y", bufs=2)
                        for j in range(2):
                            jsl = slice(j * CW, (j + 1) * CW)
                            yp = pm.tile([128, CW], dt.float32, tag="pm")
                            for k in range(KT):
                                nc.tensor.matmul(yp[:], xfT[:, k, ns],
                                                 wp_s[:, k, jsl],
                                                 start=(k == 0),
                                                 stop=(k == KT - 1))
                            nc.vector.tensor_tensor(out=y[:, jsl], in0=yp[:],
                                                    in1=bpr[:, jsl], op=OP.add)
                        s1 = fp.tile([128, 1], dt.float32, tag="s1", bufs=2)
                        scr1 = fp.tile([128, D], dt.float32, tag="scr1", bufs=2)
                        nc.scalar.activation(scr1[:], y[:], AF.Identity,
                                             accum_out=s1[:])
                        s2 = fp.tile([128, 1], dt.float32, tag="s2", bufs=2)
                        scr2 = fp.tile([128, D], dt.float32, tag="scr2", bufs=2)
                        nc.scalar.activation(scr2[:], y[:], AF.Square,
                                             accum_out=s2[:])
                        mu = fp.tile([128, 1], dt.float32, tag="mu", bufs=2)
                        nc.vector.tensor_scalar(out=mu[:], in0=s1[:],
                                                scalar1=1.0 / D, scalar2=None,
                                                op0=OP.mult)
                        msq = fp.tile([128, 1], dt.float32, tag="msq", bufs=2)
                        nc.vector.tensor_scalar(out=msq[:], in0=s2[:],
                                                scalar1=1.0 / D, scalar2=None,
                                                op0=OP.mult)
                        var = fp.tile([128, 1], dt.float32, tag="var", bufs=2)
                        nc.vector.tensor_tensor(out=var[:], in0=mu[:], in1=mu[:],
                                                op=OP.mult)
                        nc.vector.tensor_tensor(out=var[:], in0=msq[:],
                                                in1=var[:], op=OP.subtract)
                        nc.vector.tensor_scalar(out=var[:], in0=var[:],
                                                scalar1=EPS_LN, scalar2=None,
                                                op0=OP.add)
                        sd = fp.tile([128, 1], dt.float32, tag="sd", bufs=2)
                        nc.scalar.activation(sd[:], var[:], AF.Sqrt)
                        rs = fp.tile([128, 1], dt.float32, tag="rs", bufs=2)
                        nc.vector.reciprocal(rs[:], sd[:])
                        mrs = fp.tile([128, 1], dt.float32, tag="mrs", bufs=2)
                        nc.vector.tensor_tensor(out=mrs[:], in0=mu[:], in1=rs[:],
                                                op=OP.mult)
                        yn = fp.tile([128, D], dt.float32, tag="yn", bufs=2)
                        nc.vector.tensor_scalar(out=yn[:], in0=y[:],
                                                scalar1=rs[:], scalar2=mrs[:],
                                                op0=OP.mult, op1=OP.subtract)
                        nc.vector.tensor_tensor(out=yn[:], in0=yn[:], in1=gmr[:],
                                                op=OP.mult)
                        nc.vector.tensor_tensor(out=yn[:], in0=yn[:], in1=btr[:],
                                                op=OP.add)
                        og = fp.tile([128, D], dt.float32, tag="og", bufs=2)
                        nc.scalar.activation(og[:], yn[:], AF.Gelu)
                        nc.sync.dma_start(out=out_d[ns, :], in_=og[:])


    nc.compile()
    return nc


_CACHE = {}


def kernel(**inputs):
    in_maps, TG = _preprocess(**inputs)
    if TG not in _CACHE:
        _CACHE[TG] = build_program(TG)
    nc = _CACHE[TG]
    res = run_bass_kernel_spmd(nc, in_maps, list(range(NC)))
    out = np.concatenate([res.results[c]["out"] for c in range(NC)], axis=0)
    return out


if __name__ == "__main__":
    pass



# revision 1
# speedup vs baseline: 1.6385x; 1.6385x over previous
"""Distributed GATv2 message-passing kernel for 8 Trainium2 NeuronCores.

Sharding: nodes (and their incoming edges) are partitioned across the 8
cores by dst-node chunk of 1024; GATv2 weights are replicated; node
features are exchanged once per layer with a bf16 AllGather.

Per core / per layer on device:
  - xr = x_own @ Wr + br           (node-space matmul, lhsT = x^T)
  - rel_proj = rel_emb @ We + bl   (64 relations only)
  - per-edge src features gathered (transposed) with dma_gather, then
    m = X_src@Wl + onehotR@rel_proj + onehotD@xr accumulated in PSUM
  - logits = 0.2*att.m + 0.8*att.relu(m); the linear att.m term is
    computed exactly in node space via host-folded v=W.att weights
  - segment softmax + weighted aggregation via one-hot matmuls (the
    edge->node-tile assignment matrices are shipped as static data)
  - head mean + gelu + residual; bf16 chunk AllGather for next layer
Final: gate/fuse with x_text, Wp projection, layernorm, gelu.

Edges are sorted by dst and packed into 128-wide tiles grouped by
128-node dst tile; tiles-per-group is padded to a global constant so one
SPMD program serves all 8 cores (per-core differences live in the data:
gather indices and one-hot matrices).
"""
import sys
sys.path.insert(0, "/opt/trn_rl_repo")

import numpy as np
import ml_dtypes

import concourse.bass as bass
import concourse.bacc as bacc
import concourse.mybir as mybir
import concourse.tile as tile
from concourse import library_config
from concourse.bass_utils import run_bass_kernel_spmd

AF = mybir.ActivationFunctionType
OP = mybir.AluOpType
dt = mybir.dt
AX = mybir.AxisListType

N, D, E, L, H, R = 8192, 768, 16384, 3, 4, 64
NC = 8            # cores
CHN = N // NC     # 1024 nodes per core
NT = CHN // 128   # 8 node tiles per core
HD = H * D        # 3072
KT = D // 128     # 6 contraction tiles
NCH = 8           # column chunks of 384 (2 per head)
CW = HD // NCH    # 384
EPS_LN = 1e-5
DEN_EPS = 1e-6

bf16 = ml_dtypes.bfloat16


# ---------------------------------------------------------------- host prep
def _preprocess(x_text, rel_emb, Wl, bl, Wr, br, We, att, bout,
                Wg, bg, Wp, bp, gamma, beta, edge_index, edge_attr):
    src_all = np.asarray(edge_index[0], np.int64)
    dst_all = np.asarray(edge_index[1], np.int64)
    rel_all = np.asarray(edge_attr, np.int64)

    per_core = []
    max_tiles = 1
    for c in range(NC):
        sel = np.nonzero((dst_all >= c * CHN) & (dst_all < (c + 1) * CHN))[0]
        order = np.argsort(dst_all[sel], kind="stable")
        sel = sel[order]
        dloc = dst_all[sel] - c * CHN
        groups = []
        for g in range(NT):
            gsel = sel[(dloc >= g * 128) & (dloc < (g + 1) * 128)]
            groups.append(gsel)
            max_tiles = max(max_tiles, (len(gsel) + 127) // 128)
        per_core.append(groups)
    TG = max_tiles               # tiles per node-tile group (uniform)
    ET = NT * TG                 # edge tiles per core
    EP = ET * 128                # padded edges per core

    in_maps = []
    w_shared = None
    for c in range(NC):
        src_idx = np.zeros(EP, np.int16)
        oneD = np.zeros((128, ET, 128), bf16)   # [dst_local, tile, e]
        oneA = np.zeros((128, ET, 128), bf16)   # [e, tile, dst_local]
        oneR = np.zeros((R, ET, 128), bf16)     # [rel, tile, e]
        for g in range(NT):
            ge = per_core[c][g]
            for i, eidx in enumerate(ge):
                t = g * TG + i // 128
                e = i % 128
                nl = int(dst_all[eidx]) - c * CHN - g * 128
                src_idx[t * 128 + e] = int(src_all[eidx])
                oneD[nl, t, e] = 1
                oneA[e, t, nl] = 1
                oneR[int(rel_all[eidx]), t, e] = 1
        idx_w = np.tile(src_idx.reshape(EP // 16, 16).T, (8, 1)).copy()

        if w_shared is None:
            # weight tensors, cast/reshaped only (shared by all cores)
            def ktile(w):  # [768, X] -> [128, 6, X]
                return np.ascontiguousarray(
                    w.reshape(KT, 128, -1).transpose(1, 0, 2)).astype(bf16)
            wl_t = np.stack([ktile(np.asarray(Wl[l])) for l in range(L)])
            wr_t = np.stack([ktile(np.asarray(Wr[l])) for l in range(L)])
            we_t = np.stack([ktile(np.asarray(We[l])) for l in range(L)])
            relT = ktile(np.asarray(rel_emb).T)          # [128, 6, 64]
            wg_t = np.ascontiguousarray(
                np.asarray(Wg).reshape(12, 128, 1).transpose(1, 0, 2)).astype(bf16)
            wp_t = ktile(np.asarray(Wp))                 # [128, 6, 768]
            attm = np.asarray(att).reshape(L, HD)        # [L, 3072]
            att_rep = np.broadcast_to(attm[:, None, :], (L, 128, HD)).astype(bf16)
            # v = per-head fold of att into Wl / Wr (weight algebra only)
            # vl[l][d, h] = sum_j Wl[l][d, h*D+j] * att[l,h,j]
            vl = np.stack([
                np.einsum("dhj,hj->dh", np.asarray(Wl[l]).reshape(D, H, D),
                          np.asarray(att[l])) for l in range(L)])
            vr = np.stack([
                np.einsum("dhj,hj->dh", np.asarray(Wr[l]).reshape(D, H, D),
                          np.asarray(att[l])) for l in range(L)])
            vl_t = np.stack([np.ascontiguousarray(
                v.reshape(KT, 128, H).transpose(1, 0, 2)).astype(bf16) for v in vl])
            vr_t = np.stack([np.ascontiguousarray(
                v.reshape(KT, 128, H).transpose(1, 0, 2)).astype(bf16) for v in vr])
            # att.br constant per head (br is [3072])
            cbr = np.einsum("lhj,lhj->lh", np.asarray(att),
                            np.asarray(br).reshape(L, H, D))
            cbr_rep = np.broadcast_to(
                cbr[:, None, :], (L, 128, H)).astype(np.float32)
            bl_rep = np.broadcast_to(
                np.asarray(bl)[:, None, :], (L, 128, HD)).astype(np.float32)
            br_rep = np.broadcast_to(
                np.asarray(br)[:, None, :], (L, 128, HD)).astype(bf16)
            bout_rep = np.broadcast_to(
                np.asarray(bout)[:, None, :], (L, 128, D)).astype(np.float32)
            bg_rep = np.full((128, 1), float(np.asarray(bg)[0]), np.float32)
            bp_rep = np.broadcast_to(
                np.asarray(bp)[None, :], (128, D)).astype(np.float32)
            gamma_rep = np.broadcast_to(
                np.asarray(gamma)[None, :], (128, D)).astype(np.float32)
            beta_rep = np.broadcast_to(
                np.asarray(beta)[None, :], (128, D)).astype(np.float32)
            x_full_bf = np.asarray(x_text, np.float32).astype(bf16)
            w_shared = dict(
                wl=np.ascontiguousarray(wl_t), wr=np.ascontiguousarray(wr_t),
                we=np.ascontiguousarray(we_t), relT=np.ascontiguousarray(relT),
                wg=np.ascontiguousarray(wg_t), wp=np.ascontiguousarray(wp_t),
                att_rep=np.ascontiguousarray(att_rep),
                vl=vl_t, vr=vr_t, cbr=np.ascontiguousarray(cbr_rep),
                bl_rep=np.ascontiguousarray(bl_rep),
                br_rep=np.ascontiguousarray(br_rep),
                bout_rep=np.ascontiguousarray(bout_rep),
                bg_rep=bg_rep, bp_rep=np.ascontiguousarray(bp_rep),
                gamma_rep=np.ascontiguousarray(gamma_rep),
                beta_rep=np.ascontiguousarray(beta_rep),
                x_full=np.ascontiguousarray(x_full_bf),
            )
        x_own = np.ascontiguousarray(np.asarray(
            x_text[c * CHN:(c + 1) * CHN], np.float32))
        m = dict(w_shared)
        m.update(x_own=x_own, x_own_bf=x_own.astype(bf16),
                 src_idx=idx_w, oneD=oneD, oneA=oneA, oneR=oneR)
        in_maps.append(m)
    return in_maps, TG


# ---------------------------------------------------------------- device
def build_program(TG, repeat=1, abl=frozenset()):
    ET = NT * TG
    EP = ET * 128
    nc = bacc.Bacc("TRN2", target_bir_lowering=False, debug=False,
                   num_devices=NC)

    def inp(name, shape, dtype):
        return nc.dram_tensor(name, list(shape), dtype, kind="ExternalInput")

    x_full = inp("x_full", [N, D], dt.bfloat16)
    x_own = inp("x_own", [CHN, D], dt.float32)
    x_own_bf = inp("x_own_bf", [CHN, D], dt.bfloat16)
    wl_d = inp("wl", [L, 128, KT, HD], dt.bfloat16)
    wr_d = inp("wr", [L, 128, KT, HD], dt.bfloat16)
    we_d = inp("we", [L, 128, KT, HD], dt.bfloat16)
    relT_d = inp("relT", [128, KT, R], dt.bfloat16)
    wg_d = inp("wg", [128, 12, 1], dt.bfloat16)
    wp_d = inp("wp", [128, KT, D], dt.bfloat16)
    att_d = inp("att_rep", [L, 128, HD], dt.bfloat16)
    vl_d = inp("vl", [L, 128, KT, H], dt.bfloat16)
    vr_d = inp("vr", [L, 128, KT, H], dt.bfloat16)
    cbr_d = inp("cbr", [L, 128, H], dt.float32)
    blr_d = inp("bl_rep", [L, 128, HD], dt.bfloat16)
    brr_d = inp("br_rep", [L, 128, HD], dt.bfloat16)
    bor_d = inp("bout_rep", [L, 128, D], dt.float32)
    bgr_d = inp("bg_rep", [128, 1], dt.float32)
    bpr_d = inp("bp_rep", [128, D], dt.float32)
    gmr_d = inp("gamma_rep", [128, D], dt.float32)
    btr_d = inp("beta_rep", [128, D], dt.float32)
    idx_d = inp("src_idx", [128, EP // 16], dt.int16)
    oneD_d = inp("oneD", [128, ET, 128], dt.bfloat16)
    oneA_d = inp("oneA", [128, ET, 128], dt.bfloat16)
    oneR_d = inp("oneR", [R, ET, 128], dt.bfloat16)

    out_d = nc.dram_tensor("out", [CHN, D], dt.float32, kind="ExternalOutput")

    xg_chunk = nc.dram_tensor("xg_chunk", [CHN, D], dt.bfloat16)
    xc_dram = nc.dram_tensor("xc_dram", [CHN, D], dt.float32)
    xg_full = nc.dram_tensor("xg_full", [N, D], dt.bfloat16,
                             addr_space="Shared")
    xf_dram = nc.dram_tensor("xf_dram", [CHN, D], dt.bfloat16)

    x_own_t = x_own.rearrange("(t p) d -> p t d", p=128)
    xc_dram_t = xc_dram.rearrange("(t p) d -> p t d", p=128)

    with tile.TileContext(nc) as tc:
        nc.gpsimd.load_library(library_config.attnmlp)
        with tc.tile_pool(name="persist", bufs=1) as pp, \
             tc.tile_pool(name="scr", bufs=2) as sp, \
             tc.tile_pool(name="pm", bufs=3, space="PSUM") as pm, \
             tc.tile_pool(name="pt", bufs=2, space="PSUM") as pt, \
             tc.tile_pool(name="pagg", bufs=2, space="PSUM") as pagg:

            idxs = pp.tile([128, EP // 16], dt.int16)
            nc.sync.dma_start(out=idxs[:], in_=idx_d[:])

            for _rep in range(repeat):
                with tc.tile_pool(name="work", bufs=1) as wk:
                  for l in range(L):
                    XF = x_full if l == 0 else xg_full
                    XCSRC = x_own_bf if l == 0 else xg_chunk
                    XMST = x_own_t if l == 0 else xc_dram_t

                    wl_s = wk.tile([128, KT, HD], dt.bfloat16, tag="wl_s")
                    nc.sync.dma_start(out=wl_s[:], in_=wl_d[l])
                    wr_s = wk.tile([128, KT, HD], dt.bfloat16, tag="wr_s")
                    nc.sync.dma_start(out=wr_s[:], in_=wr_d[l])
                    attr = wk.tile([128, HD], dt.bfloat16, tag="attr")
                    nc.sync.dma_start(out=attr[:], in_=att_d[l])
                    brr = wk.tile([128, HD], dt.bfloat16, tag="brr")
                    nc.sync.dma_start(out=brr[:], in_=brr_d[l])
                    blr = wk.tile([128, HD], dt.bfloat16, tag="blr")
                    nc.sync.dma_start(out=blr[:], in_=blr_d[l])
                    bor = wk.tile([128, D], dt.float32, tag="bor")
                    nc.sync.dma_start(out=bor[:], in_=bor_d[l])
                    vl_s = wk.tile([128, KT, H], dt.bfloat16, tag="vl_s")
                    nc.sync.dma_start(out=vl_s[:], in_=vl_d[l])
                    vr_s = wk.tile([128, KT, H], dt.bfloat16, tag="vr_s")
                    nc.sync.dma_start(out=vr_s[:], in_=vr_d[l])
                    cbr_s = wk.tile([128, H], dt.float32, tag="cbr_s")
                    nc.sync.dma_start(out=cbr_s[:], in_=cbr_d[l])

                    # blmean = 0.25 * sum_h bl[h]
                    blm = wk.tile([128, D], dt.float32, tag="blm")
                    nc.vector.tensor_tensor(out=blm[:], in0=blr[:, 0:D],
                                            in1=blr[:, D:2 * D], op=OP.add)
                    nc.vector.tensor_tensor(out=blm[:], in0=blm[:],
                                            in1=blr[:, 2 * D:3 * D], op=OP.add)
                    nc.vector.tensor_tensor(out=blm[:], in0=blm[:],
                                            in1=blr[:, 3 * D:4 * D], op=OP.add)
                    nc.vector.tensor_scalar(out=blm[:], in0=blm[:], scalar1=0.25,
                                            scalar2=None, op0=OP.mult)

                    # rel_proj' = rel_emb @ We + bl  -> [64, 3072] bf16
                    relT_s = wk.tile([128, KT, R], dt.bfloat16, tag="relT_s")
                    nc.sync.dma_start(out=relT_s[:], in_=relT_d[:])
                    relp = wk.tile([R, HD], dt.bfloat16, tag="relp")
                    for ch in range(NCH):
                        sl = slice(ch * CW, (ch + 1) * CW)
                        wech = wk.tile([128, KT, CW], dt.bfloat16, tag="wech")
                        nc.sync.dma_start(out=wech[:], in_=we_d[l][:, :, sl])
                        ps = pm.tile([128, CW], dt.float32, tag="pm")
                        for k in range(KT):
                            nc.tensor.matmul(ps[:R, :], relT_s[:, k, :],
                                             wech[:, k, :],
                                             start=(k == 0), stop=(k == KT - 1))
                        nc.vector.tensor_tensor(out=relp[:, sl], in0=ps[:R, :],
                                                in1=blr[:R, sl], op=OP.add)
                    # a_e[r, h] = att_h . relp'[r, h-block]
                    ae = wk.tile([R, H], dt.float32, tag="ae")
                    ae_bf = wk.tile([R, H], dt.bfloat16, tag="ae_bf")
                    for h in range(H):
                        hb = slice(h * D, (h + 1) * D)
                        scr_ae = sp.tile([R, D], dt.bfloat16, tag="scr_ae")
                        nc.vector.tensor_tensor(out=scr_ae[:], in0=relp[:, hb],
                                                in1=attr[:R, hb], op=OP.mult)
                        nc.vector.tensor_reduce(out=ae[:, h:h + 1], in_=scr_ae[:],
                                                axis=AX.X, op=OP.add)
                    nc.vector.tensor_copy(ae_bf[:], ae[:])

                    for g in range(NT):
                        gs = slice(g * 128, (g + 1) * 128)
                        # x^T for this node tile
                        xT = wk.tile([128, KT, 128], dt.bfloat16, tag="xT",
                                     bufs=2)
                        for k in range(KT):
                            nc.sync.dma_start(
                                out=xT[:, k, :],
                                in_=XCSRC[gs, k * 128:(k + 1) * 128],
                                transpose=True)
                        # xr_g = x_g @ Wr + br ; a_r_g = x_g @ vr + cbr
                        xr = wk.tile([128, HD], dt.bfloat16, tag="xr", bufs=2)
                        if "ph1" in abl:
                            nc.vector.memset(xr[:], 0.0)
                        for ch in (range(NCH) if "ph1" not in abl else []):
                            sl = slice(ch * CW, (ch + 1) * CW)
                            ps = pm.tile([128, CW], dt.float32, tag="pm")
                            for k in range(KT):
                                nc.tensor.matmul(ps[:], xT[:, k, :],
                                                 wr_s[:, k, sl],
                                                 start=(k == 0), stop=(k == KT - 1))
                            nc.vector.tensor_tensor(out=xr[:, sl], in0=ps[:],
                                                    in1=brr[:, sl], op=OP.add)
                        arp = pt.tile([128, H], dt.float32, tag="pt")
                        for k in range(KT):
                            nc.tensor.matmul(arp[:], xT[:, k, :], vr_s[:, k, :],
                                             start=(k == 0), stop=(k == KT - 1))
                        art = sp.tile([128, H], dt.float32, tag="art")
                        nc.vector.tensor_tensor(out=art[:], in0=arp[:],
                                                in1=cbr_s[:], op=OP.add)
                        ar_bf = wk.tile([128, H], dt.bfloat16, tag="ar_bf",
                                        bufs=2)
                        nc.vector.tensor_copy(ar_bf[:], art[:])

                        # per-group static one-hots
                        ts0 = g * TG
                        oneDg = wk.tile([128, TG, 128], dt.bfloat16, tag="oneDg",
                                        bufs=2)
                        nc.sync.dma_start(out=oneDg[:],
                                          in_=oneD_d[:, ts0:ts0 + TG, :])
                        oneAg = wk.tile([128, TG, 128], dt.bfloat16, tag="oneAg",
                                        bufs=2)
                        nc.sync.dma_start(out=oneAg[:],
                                          in_=oneA_d[:, ts0:ts0 + TG, :])
                        oneRg = wk.tile([R, TG, 128], dt.bfloat16, tag="oneRg",
                                        bufs=2)
                        nc.sync.dma_start(out=oneRg[:],
                                          in_=oneR_d[:, ts0:ts0 + TG, :])

                        e0 = g * TG * 128
                        xsT = wk.tile([128, KT, TG * 128], dt.bfloat16,
                                      tag="xsT", bufs=2)
                        if "gather" not in abl:
                            nc.gpsimd.dma_gather(
                                xsT[:], XF[:],
                                idxs[:, e0 // 16:(e0 + TG * 128) // 16],
                                num_idxs=TG * 128, num_idxs_reg=TG * 128,
                                elem_size=D, transpose=True)
                        else:
                            nc.sync.dma_start(
                                out=xsT.rearrange("p a b -> p (a b)"),
                                in_=XF[0:TG * 128, :].rearrange(
                                    "(p a) d -> p (a d)", p=128))
                        xl_sb = wk.tile([128, TG, HD], dt.bfloat16, tag="xl_sb")
                        ex_sb = wk.tile([128, TG, H], dt.bfloat16, tag="ex_sb")
                        exf = wk.tile([128, TG, H], dt.float32, tag="exf")
                        for kt in range(TG):
                            es = slice(kt * 128, (kt + 1) * 128)
                            # linear logit part: x_src@vl + a_r[dst] + a_e[rel]
                            aps = pt.tile([128, H], dt.float32, tag="pt")
                            for k in range(KT):
                                nc.tensor.matmul(aps[:], xsT[:, k, es],
                                                 vl_s[:, k, :],
                                                 start=(k == 0), stop=False)
                            nc.tensor.matmul(aps[:], oneDg[:, kt, :],
                                             ar_bf[:], start=False, stop=False)
                            nc.tensor.matmul(aps[:], oneRg[:, kt, :], ae_bf[:],
                                             start=False, stop=True)
                            racc = sp.tile([128, 2, H], dt.float32, tag="racc")
                            nc.vector.memset(racc[:], 0.0)
                            for ch in (range(NCH) if "medge" not in abl else []):
                                h, j = ch // 2, ch % 2
                                sl = slice(ch * CW, (ch + 1) * CW)
                                mp = pm.tile([128, CW], dt.float32, tag="pm")
                                for k in range(KT):
                                    nc.tensor.matmul(mp[:], xsT[:, k, es],
                                                     wl_s[:, k, sl],
                                                     start=(k == 0),
                                                     stop=(k == KT - 1))
                                nc.vector.tensor_copy(xl_sb[:, kt, sl], mp[:])
                                nc.tensor.matmul(mp[:], oneRg[:, kt, :],
                                                 relp[:, sl], start=False,
                                                 stop=False)
                                nc.tensor.matmul(mp[:], oneDg[:, kt, :],
                                                 xr[:, sl], start=False,
                                                 stop=True)
                                rt = sp.tile([128, CW], dt.bfloat16, tag="rt")
                                nc.scalar.activation(rt[:], mp[:], AF.Relu)
                                sc = sp.tile([128, CW], dt.bfloat16, tag="sc")
                                nc.vector.tensor_tensor(out=sc[:], in0=rt[:],
                                                        in1=attr[:, sl],
                                                        op=OP.mult)
                                nc.vector.tensor_reduce(
                                    out=racc[:, j, h:h + 1], in_=sc[:],
                                    axis=AX.X, op=OP.add)
                            if "medge" in abl:
                                nc.vector.memset(
                                    xl_sb.rearrange("p a b -> p (a b)"), 0.0)
                            # logits = 0.2*aps + 0.8*(racc_j0 + racc_j1)
                            lg = sp.tile([128, H], dt.float32, tag="lg")
                            nc.vector.tensor_tensor(
                                out=lg[:], in0=racc[:, 0, :], in1=racc[:, 1, :],
                                op=OP.add)
                            nc.vector.tensor_scalar(out=lg[:], in0=lg[:],
                                                    scalar1=0.8, scalar2=None,
                                                    op0=OP.mult)
                            lg2 = sp.tile([128, H], dt.float32, tag="lg2")
                            nc.vector.tensor_scalar(out=lg2[:], in0=aps[:],
                                                    scalar1=0.2, scalar2=None,
                                                    op0=OP.mult)
                            nc.vector.tensor_tensor(out=lg[:], in0=lg[:],
                                                    in1=lg2[:], op=OP.add)
                            nc.scalar.activation(exf[:, kt, :], lg[:], AF.Exp)
                            nc.vector.tensor_copy(ex_sb[:, kt, :], exf[:, kt, :])

                        # segment softmax pieces
                        dn = pt.tile([128, H], dt.float32, tag="pt")
                        for kt in range(TG):
                            nc.tensor.matmul(dn[:], oneAg[:, kt, :],
                                             ex_sb[:, kt, :],
                                             start=(kt == 0), stop=(kt == TG - 1))
                        dn4 = sp.tile([128, H], dt.float32, tag="dn4")
                        nc.vector.tensor_scalar(out=dn4[:], in0=dn[:],
                                                scalar1=4.0, scalar2=DEN_EPS,
                                                op0=OP.mult, op1=OP.add)
                        has = sp.tile([128, 1], dt.float32, tag="has")
                        nc.vector.tensor_scalar(out=has[:], in0=dn4[:, 0:1],
                                                scalar1=2.0 * DEN_EPS,
                                                scalar2=None, op0=OP.is_gt)
                        rden = sp.tile([128, H], dt.float32, tag="rden")
                        nc.vector.reciprocal(rden[:], dn4[:])
                        rden_bf = sp.tile([128, H], dt.bfloat16, tag="rden_bf")
                        nc.vector.tensor_copy(rden_bf[:], rden[:])

                        A_sb = wk.tile([128, TG, H, 128], dt.bfloat16, tag="A_sb")
                        if "agg" in abl:
                            nc.vector.memset(
                                A_sb.rearrange("p a b c -> p (a b c)"), 0.0)
                        for kt in (range(TG) if "agg" not in abl else []):
                            re = pt.tile([128, H], dt.float32, tag="pt")
                            nc.tensor.matmul(re[:], oneDg[:, kt, :], rden_bf[:],
                                             start=True, stop=True)
                            re_f = sp.tile([128, H], dt.float32, tag="re_f")
                            nc.vector.tensor_copy(re_f[:], re[:])
                            for h in range(H):
                                nc.vector.tensor_scalar(
                                    out=A_sb[:, kt, h, :], in0=oneAg[:, kt, :],
                                    scalar1=exf[:, kt, h:h + 1],
                                    scalar2=re_f[:, h:h + 1],
                                    op0=OP.mult, op1=OP.mult)

                        # aggregate + head mean (0.25 folded via 4*den)
                        xcg = sp.tile([128, D], dt.float32, tag="escr2", bufs=2)
                        nc.sync.dma_start(out=xcg[:], in_=XMST[:, g, :])
                        for j in range(2):
                            jsl = slice(j * CW, (j + 1) * CW)
                            ag = pagg.tile([128, CW], dt.float32, tag="pagg")
                            if "aggmm" in abl:
                                nc.vector.memset(ag[:], 0.0)
                            first = True
                            for kt in (range(TG) if "aggmm" not in abl else []):
                                for h in range(H):
                                    nc.tensor.matmul(
                                        ag[:], A_sb[:, kt, h, :],
                                        xl_sb[:, kt, h * D + j * CW:
                                              h * D + (j + 1) * CW],
                                        start=first,
                                        stop=(kt == TG - 1 and h == H - 1))
                                    first = False
                            u = sp.tile([128, CW], dt.float32, tag="escr", bufs=5)
                            nc.vector.tensor_tensor(out=u[:], in0=ag[:],
                                                    in1=bor[:, jsl], op=OP.add)
                            bh = sp.tile([128, CW], dt.float32, tag="escr", bufs=5)
                            nc.vector.tensor_scalar(out=bh[:], in0=blm[:, jsl],
                                                    scalar1=has[:],
                                                    scalar2=None, op0=OP.mult)
                            nc.vector.tensor_tensor(out=u[:], in0=u[:],
                                                    in1=bh[:], op=OP.add)
                            gl = sp.tile([128, CW], dt.float32, tag="escr", bufs=5)
                            nc.scalar.activation(gl[:], u[:], AF.Gelu)
                            xn = sp.tile([128, CW], dt.float32, tag="escr", bufs=5)
                            nc.vector.tensor_tensor(out=xn[:], in0=gl[:],
                                                    in1=xcg[:, jsl], op=OP.add)
                            nc.sync.dma_start(out=xc_dram_t[:, g, jsl], in_=xn[:])
                            xnb = sp.tile([128, CW], dt.bfloat16, tag="xnb")
                            nc.vector.tensor_copy(xnb[:], xn[:])
                            nc.sync.dma_start(
                                out=xg_chunk[gs, jsl], in_=xnb[:])

                    if l < L - 1 and "ag" not in abl:
                        nc.gpsimd.collective_compute(
                            "AllGather", OP.bypass,
                            ins=[xg_chunk[:]], outs=[xg_full[:]],
                            replica_groups=[list(range(NC))])

                # ------------- final: gate, fuse, project, layernorm, gelu
                with tc.tile_pool(name="fin", bufs=1) as fp:
                    xgT = fp.tile([128, KT, CHN], dt.bfloat16, tag="xgT")
                    for k in range(KT):
                        nc.sync.dma_start(out=xgT[:, k, :],
                                          in_=xg_chunk[:, k * 128:(k + 1) * 128],
                                          transpose=True)
                    xtT = fp.tile([128, KT, CHN], dt.bfloat16, tag="xtT")
                    for k in range(KT):
                        nc.sync.dma_start(out=xtT[:, k, :],
                                          in_=x_own_bf[:, k * 128:(k + 1) * 128],
                                          transpose=True)
                    wg_s = fp.tile([128, 12, 1], dt.bfloat16, tag="wg_s")
                    nc.sync.dma_start(out=wg_s[:], in_=wg_d[:])
                    bgr = fp.tile([128, 1], dt.float32, tag="bgr")
                    nc.sync.dma_start(out=bgr[:], in_=bgr_d[:])

                    for nt in range(NT):
                        ns = slice(nt * 128, (nt + 1) * 128)
                        pg = pt.tile([128, 1], dt.float32, tag="pt")
                        for k in range(KT):
                            nc.tensor.matmul(pg[:], xtT[:, k, ns], wg_s[:, k, :],
                                             start=(k == 0), stop=False)
                        for k in range(KT):
                            nc.tensor.matmul(pg[:], xgT[:, k, ns],
                                             wg_s[:, KT + k, :],
                                             start=False, stop=(k == KT - 1))
                        alph = fp.tile([128, 1], dt.float32, tag="alph", bufs=2)
                        nc.scalar.activation(alph[:], pg[:], AF.Sigmoid,
                                             bias=bgr[:])
                        xct = fp.tile([128, D], dt.float32, tag="xct", bufs=2)
                        nc.sync.dma_start(out=xct[:], in_=xc_dram_t[:, nt, :])
                        xtt = fp.tile([128, D], dt.float32, tag="xtt", bufs=2)
                        nc.sync.dma_start(out=xtt[:], in_=x_own_t[:, nt, :])
                        dif = fp.tile([128, D], dt.float32, tag="dif", bufs=2)
                        nc.vector.tensor_tensor(out=dif[:], in0=xct[:],
                                                in1=xtt[:], op=OP.subtract)
                        nc.vector.tensor_scalar(out=dif[:], in0=dif[:],
                                                scalar1=alph[:], scalar2=None,
                                                op0=OP.mult)
                        nc.vector.tensor_tensor(out=dif[:], in0=dif[:],
                                                in1=xtt[:], op=OP.add)
                        dif_bf = fp.tile([128, D], dt.bfloat16, tag="dif_bf", bufs=2)
                        nc.vector.tensor_copy(dif_bf[:], dif[:])
                        nc.sync.dma_start(out=xf_dram[ns, :], in_=dif_bf[:])

                    xfT = fp.tile([128, KT, CHN], dt.bfloat16, tag="xgT")
                    for k in range(KT):
                        nc.sync.dma_start(out=xfT[:, k, :],
                                          in_=xf_dram[:, k * 128:(k + 1) * 128],
                                          transpose=True)
                    wp_s = fp.tile([128, KT, D], dt.bfloat16, tag="wp_s")
                    nc.sync.dma_start(out=wp_s[:], in_=wp_d[:])
                    bpr = fp.tile([128, D], dt.float32, tag="bpr")
                    nc.sync.dma_start(out=bpr[:], in_=bpr_d[:])
                    gmr = fp.tile([128, D], dt.float32, tag="gmr")
                    nc.sync.dma_start(out=gmr[:], in_=gmr_d[:])
                    btr = fp.tile([128, D], dt.float32, tag="btr")
                    nc.sync.dma_start(out=btr[:], in_=btr_d[:])

                    for nt in range(NT):
                        ns = slice(nt * 128, (nt + 1) * 128)
                        y = fp.tile([128, D], dt.float32, tag="y", bufs=2)
                        for j in range(2):
                            jsl = slice(j * CW, (j + 1) * CW)
                            yp = pm.tile([128, CW], dt.float32, tag="pm")
                            for k in range(KT):
                                nc.tensor.matmul(yp[:], xfT[:, k, ns],
                                                 wp_s[:, k, jsl],
                                                 start=(k == 0),
                                                 stop=(k == KT - 1))
                            nc.vector.tensor_tensor(out=y[:, jsl], in0=yp[:],
                                                    in1=bpr[:, jsl], op=OP.add)
                        s1 = fp.tile([128, 1], dt.float32, tag="s1", bufs=2)
                        scr1 = fp.tile([128, D], dt.float32, tag="scr1", bufs=2)
                        nc.scalar.activation(scr1[:], y[:], AF.Identity,
                                             accum_out=s1[:])
                        s2 = fp.tile([128, 1], dt.float32, tag="s2", bufs=2)
                        scr2 = fp.tile([128, D], dt.float32, tag="scr2", bufs=2)
                        nc.scalar.activation(scr2[:], y[:], AF.Square,
                                             accum_out=s2[:])
                        mu = fp.tile([128, 1], dt.float32, tag="mu", bufs=2)
                        nc.vector.tensor_scalar(out=mu[:], in0=s1[:],
                                                scalar1=1.0 / D, scalar2=None,
                                                op0=OP.mult)
                        msq = fp.tile([128, 1], dt.float32, tag="msq", bufs=2)
                        nc.vector.tensor_scalar(out=msq[:], in0=s2[:],
                                                scalar1=1.0 / D, scalar2=None,
                                                op0=OP.mult)
                        var = fp.tile([128, 1], dt.float32, tag="var", bufs=2)
                        nc.vector.tensor_tensor(out=var[:], in0=mu[:], in1=mu[:],
                                                op=OP.mult)
                        nc.vector.tensor_tensor(out=var[:], in0=msq[:],
                                                in1=var[:], op=OP.subtract)
                        nc.vector.tensor_scalar(out=var[:], in0=var[:],
                                                scalar1=EPS_LN, scalar2=None,
                                                op0=OP.add)
                        sd = fp.tile([128, 1], dt.float32, tag="sd", bufs=2)
                        nc.scalar.activation(sd[:], var[:], AF.Sqrt)
                        rs = fp.tile([128, 1], dt.float32, tag="rs", bufs=2)
                        nc.vector.reciprocal(rs[:], sd[:])
                        mrs = fp.tile([128, 1], dt.float32, tag="mrs", bufs=2)
                        nc.vector.tensor_tensor(out=mrs[:], in0=mu[:], in1=rs[:],
                                                op=OP.mult)
                        yn = fp.tile([128, D], dt.float32, tag="yn", bufs=2)
                        nc.vector.tensor_scalar(out=yn[:], in0=y[:],
                                                scalar1=rs[:], scalar2=mrs[:],
                                                op0=OP.mult, op1=OP.subtract)
                        nc.vector.tensor_tensor(out=yn[:], in0=yn[:], in1=gmr[:],
                                                op=OP.mult)
                        nc.vector.tensor_tensor(out=yn[:], in0=yn[:], in1=btr[:],
                                                op=OP.add)
                        og = fp.tile([128, D], dt.float32, tag="og", bufs=2)
                        nc.scalar.activation(og[:], yn[:], AF.Gelu)
                        nc.sync.dma_start(out=out_d[ns, :], in_=og[:])


    nc.compile()
    return nc


_CACHE = {}


def kernel(**inputs):
    in_maps, TG = _preprocess(**inputs)
    if TG not in _CACHE:
        _CACHE[TG] = build_program(TG)
    nc = _CACHE[TG]
    res = run_bass_kernel_spmd(nc, in_maps, list(range(NC)))
    out = np.concatenate([res.results[c]["out"] for c in range(NC)], axis=0)
    return out


if __name__ == "__main__":
    pass

